# revision 11
# baseline (speedup 1.0000x reference)
"""Trainium2 Bass kernel for a custom-activation LSTM cell.

  gates = (x @ w_ih.T + b_ih) + (h @ w_hh.T + b_hh)   # [B, 4H], gate order f,i,ic,o
  ft, it, ot = sigmoid(...), i_cands = sin(ic_in)
  ct = c*ft + sin(ic_in)*it ; ht = sigmoid(o_in)*sin(ct)

Sharding: each of the 8 cores computes the SAME 256-wide slice of H for all
four gates; core owns columns [core*256, (core+1)*256) of ht/ct — no
cross-core communication.

Matmul precision/speed: fp8(e4m3) DoubleRowSwInterleave runs the PE at 2x the
fp32r stream rate. Raw fp8 error (~3e-2) exceeds the 2e-2 gate, so residual
compensation is used (measured 9.3e-4):

  x ~= (X + S)/32,  W ~= (W' + R)/256   with X=e4m3(32x), S=e4m3(32x - X),
                                             W'=e4m3(256w), R=e4m3(256w - W')
  x@w ~= (X@W' + S@W' + X@R) / 8192     (all terms share PSUM scale 8192)

Per m-tile: 12 main pair-stationaries (W'[2k],W'[2k+1]) + 24 correction
stationaries (W'[k],R[k]) paired against moving (S[k],X[k]).

The measured bottleneck is the PE stationary (LdWeights) cost: ~177 ns per
weight change, serialized with the ~51 ns stream. The hardware elides the
reload when consecutive matmuls share the stationary AP, so each stationary
is reused across C=4 batch chunks: per-iteration loads drop 4x
(2304 -> 576 per core) for a predicted ~0.45x of the naive fp8 time.
Structure per group of 4 chunks: for each m-tile, stream its weight window
from DRAM (m-major layout, contiguous), run 36 stationaries x 4 chunks into
4 PSUM banks (8-bank ring = current + next gate), drain each gate to bf16
SBUF tiles immediately, then finish ct/ht per chunk on DVE/ACT.

sigmoid(z+b) = 0.5*tanh((0.5/kappa)*psum + 0.5b)+0.5; the sin gate wraps the
kappa-scaled psum on the DVE (shift kappa*b, bound kappa*pi) and descales in
the ACT (scale 1/kappa). Tanh and Sin come from ONE ACT table set
(silu_and_others, forced by a leading Silu) to avoid ~2.7us table switches.
"""

import numpy as np
import ml_dtypes

import concourse.bass as bass
import concourse.tile as tile
from concourse import bacc, mybir
from concourse.bass_utils import run_bass_kernel_spmd

# Problem shapes (hardcoded per the harness contract).
B, IN, H = 4096, 1024, 2048
NCORES = 8
P = 128
SH = H // NCORES          # 256  H-slice per core
G = 4 * SH                # 1024 gate rows per core (f,i,ic,o x 256)
MT = G // P               # 8 m-tiles: [f0 i0 ic0 o0 | f1 i1 ic1 o1]
KX = IN // P              # 8 k-tiles from x
KH = H // P               # 16 k-tiles from h
KP = (KX + KH) // 2       # 12 main k-tile pairs
KT = KX + KH              # 24 correction k-tiles
NB = 512                  # batch chunk (PSUM bank = 512 fp32)
NBCH = B // NB            # 8 chunks
C = 4                     # chunks sharing each stationary load
NG = NBCH // C            # 2 chunk groups

AQ = 32.0                 # activation pre-scale into fp8
WQ = 256.0                # weight pre-scale into fp8
KAPPA = AQ * WQ           # PSUM scale of every matmul term

F32 = mybir.dt.float32
BF16 = mybir.dt.bfloat16
E4 = mybir.dt.float8e4
E4NP = ml_dtypes.float8_e4m3   # numpy dtype matching mybir.dt.np(float8e4)
ACT = mybir.ActivationFunctionType
SWIL = mybir.MatmulPerfMode.DoubleRowSwInterleave

_MODULES: dict[int, "bacc.Bacc"] = {}


def _build_module(repeats: int = 1, lead_silu: bool = True,
                  internal_io: bool = False) -> "bacc.Bacc":
    """Build + compile the per-core Bass module.

    repeats > 1 wraps the whole compute in a hardware loop (used only for
    timing: the per-iteration device time is (T(R) - T(1)) / (R - 1))."""
    nc = bacc.Bacc("TRN2", target_bir_lowering=False, debug=False,
                   num_devices=NCORES)

    kin = "Internal" if internal_io else "ExternalInput"
    kout = "Internal" if internal_io else "ExternalOutput"

    # Acts: rows (k-tile, plane, partition); plane 0 = S residual, 1 = X main.
    xq = nc.dram_tensor("xq", [KX * 2 * P, B], E4, kind=kin).ap()
    hq = nc.dram_tensor("hq", [KH * 2 * P, B], E4, kind=kin).ap()
    cT = nc.dram_tensor("cT", [SH, B], F32, kind=kin).ap()
    # Weights m-major so each m-tile's window is one contiguous DMA:
    # wm rows (m, p), cols (kp, 256) swil-interleaved main pairs;
    # wc rows (m, p), cols (kt, 256) swil-interleaved (W'[kt], R[kt]).
    wm = nc.dram_tensor("wm", [MT * P, KP * 256], E4, kind=kin).ap()
    wc = nc.dram_tensor("wc", [MT * P, KT * 256], E4, kind=kin).ap()
    biasd = nc.dram_tensor("biasd", [P, MT], F32, kind="ExternalInput").ap()
    htT = nc.dram_tensor("htT", [SH, B], F32, kind=kout).ap()
    ctT = nc.dram_tensor("ctT", [SH, B], F32, kind=kout).ap()

    xq4 = xq.rearrange("(ko two p) b -> p ko two b", p=P, two=2)
    hq4 = hq.rearrange("(ko two p) b -> p ko two b", p=P, two=2)
    cT3 = cT.rearrange("(po p) b -> p po b", p=P)       # [128, 2, B]
    wm3 = wm.rearrange("(m p) f -> m p f", p=P)
    wc3 = wc.rearrange("(m p) f -> m p f", p=P)
    htT3 = htT.rearrange("(po p) b -> p po b", p=P)
    ctT3 = ctT.rearrange("(po p) b -> p po b", p=P)

    PI, TWO_PI = float(np.pi), float(2 * np.pi)
    MUL, ADD = mybir.AluOpType.mult, mybir.AluOpType.add

    with tile.TileContext(nc) as tc:
        with (
            tc.tile_pool(name="bpool", bufs=1) as bpool,
            tc.tile_pool(name="wwin", bufs=2) as wwin,
            tc.tile_pool(name="apool", bufs=C + 1) as apool,
            tc.tile_pool(name="gpool", bufs=C + 1) as gpool,
            tc.tile_pool(name="wpool", bufs=2) as wpool,
            tc.tile_pool(name="opool", bufs=2) as opool,
            tc.tile_pool(name="pspool", bufs=8, space="PSUM") as pspool,
        ):
            bias_sb = bpool.tile([P, MT], F32)
            nc.sync.dma_start(out=bias_sb, in_=biasd)

            # Dummy Silu: forces the ACT table loader to pick the
            # silu_and_others set (contains BOTH Tanh and Sin) once.
            if lead_silu:
                dummy = bpool.tile([P, 1], F32)
                nc.vector.memset(dummy, 0.0)
                nc.scalar.activation(dummy, dummy, ACT.Silu)

            def body():
                for g in range(NG):
                    xcs, hcs, ccs = [], [], []
                    for ci in range(C):
                        nb = g * C + ci
                        bsl = bass.ds(nb * NB, NB)
                        xc = apool.tile([P, KX, 2, NB], E4, tag="xc")
                        nc.sync.dma_start(out=xc, in_=xq4[:, :, :, bsl])
                        xcs.append(xc)
                        hc = apool.tile([P, KH, 2, NB], E4, tag="hc")
                        nc.sync.dma_start(out=hc, in_=hq4[:, :, :, bsl])
                        hcs.append(hc)
                        cc = apool.tile([P, 2, NB], F32, tag="cc")
                        nc.sync.dma_start(out=cc, in_=cT3[:, :, bsl])
                        ccs.append(cc)

                    for ph in range(2):  # H-slice half (two 128-row m-tiles)
                        gates = [[None] * C for _ in range(4)]
                        for gi in range(4):  # f, i, ic, o
                            mcol = gi + 4 * ph
                            wmw = wwin.tile([P, KP * 256], E4, tag="wmw")
                            nc.sync.dma_start(out=wmw, in_=wm3[mcol])
                            wcw = wwin.tile([P, KT * 256], E4, tag="wcw")
                            nc.sync.dma_start(out=wcw, in_=wc3[mcol])
                            pts = []
                            for ci in range(C):
                                pt = pspool.tile([P, NB], F32, tag="ps")
                                pts.append(pt)
                            # main X@W': each pair stationary drives C chunks
                            for kp in range(KP):
                                lw = wmw[:, bass.ds(kp * 256, 256)]
                                for ci in range(C):
                                    rhs = (xcs[ci][:, 2 * kp:2 * kp + 2, 1, :]
                                           if kp < KX // 2 else
                                           hcs[ci][:, 2 * (kp - KX // 2):
                                                   2 * (kp - KX // 2) + 2, 1, :])
                                    nc.tensor.matmul(
                                        pts[ci], lhsT=lw, rhs=rhs,
                                        start=(kp == 0), stop=False,
                                        perf_mode=SWIL)
                            # correction S@W' + X@R per k-tile
                            for kt in range(KT):
                                lw = wcw[:, bass.ds(kt * 256, 256)]
                                for ci in range(C):
                                    rhs = (xcs[ci][:, kt, :, :] if kt < KX
                                           else hcs[ci][:, kt - KX, :, :])
                                    nc.tensor.matmul(
                                        pts[ci], lhsT=lw, rhs=rhs,
                                        start=False, stop=(kt == KT - 1),
                                        perf_mode=SWIL)
                            # drain this gate's 4 PSUM banks to bf16 SBUF
                            bcol = bias_sb[:, mcol:mcol + 1]
                            for ci in range(C):
                                gt = gpool.tile([P, NB], BF16, tag=f"g{gi}")
                                if gi == 2:
                                    gw = wpool.tile([P, NB], F32, tag="gw")
                                    nc.vector.add_range_wrap(
                                        gw, pts[ci], bcol,
                                        KAPPA * PI, KAPPA * TWO_PI)
                                    nc.scalar.activation(gt, gw, ACT.Sin,
                                                         scale=1.0 / KAPPA)
                                else:
                                    nc.scalar.activation(gt, pts[ci], ACT.Tanh,
                                                         bias=bcol,
                                                         scale=0.5 / KAPPA)
                                    nc.vector.tensor_scalar(gt, gt, 0.5, 0.5,
                                                            MUL, ADD)
                                gates[gi][ci] = gt

                        for ci in range(C):
                            nb = g * C + ci
                            bsl = bass.ds(nb * NB, NB)
                            ctn = opool.tile([P, NB], F32, tag="ctn")
                            tmp = opool.tile([P, NB], F32, tag="tmp")
                            nc.vector.tensor_mul(ctn, ccs[ci][:, ph, :],
                                                 gates[0][ci])
                            nc.vector.tensor_mul(tmp, gates[2][ci],
                                                 gates[1][ci])
                            nc.vector.tensor_add(ctn, ctn, tmp)
                            cw = opool.tile([P, NB], F32, tag="cw")
                            nc.vector.add_range_wrap(cw, ctn, 0.0, PI, TWO_PI)
                            sct = opool.tile([P, NB], F32, tag="sct")
                            nc.scalar.activation(sct, cw, ACT.Sin)
                            htn = opool.tile([P, NB], F32, tag="htn")
                            nc.vector.tensor_mul(htn, gates[3][ci], sct)
                            nc.sync.dma_start(out=ctT3[:, ph, bsl], in_=ctn)
                            nc.sync.dma_start(out=htT3[:, ph, bsl], in_=htn)

            if repeats == 1:
                body()
            else:
                with tc.For_i(0, repeats, 1):
                    body()

            if internal_io:
                done = nc.dram_tensor("done", [P, MT], F32,
                                      kind="ExternalOutput").ap()
                dtile = bpool.tile([P, MT], F32)
                nc.vector.tensor_copy(dtile, bias_sb)
                nc.sync.dma_start(out=done, in_=dtile)

    nc.compile()
    return nc


def _get_module(repeats: int = 1) -> "bacc.Bacc":
    if repeats not in _MODULES:
        _MODULES[repeats] = _build_module(repeats)
    return _MODULES[repeats]


def _quant_pair(a: np.ndarray, scale: float):
    """Return (residual, main) e4m3 planes of scale*a: X = e4(scale*a),
    S = e4(scale*a - X)."""
    sa = np.asarray(a, np.float32) * np.float32(scale)
    X = sa.astype(E4NP)
    S = (sa - X.astype(np.float32)).astype(E4NP)
    return S, X


def _interleave(p0: np.ndarray, p1: np.ndarray, kt: int):
    """[kt*128, N] planes -> [kt*2*128, N] rows ordered (k, plane, p).
    Acts: p0 = S residual, p1 = X main."""
    n = p0.shape[1]
    out = np.empty((kt, 2, P, n), E4NP)
    out[:, 0] = p0.reshape(kt, P, n)
    out[:, 1] = p1.reshape(kt, P, n)
    return np.ascontiguousarray(out.reshape(kt * 2 * P, n))


def make_in_maps(x, h, c, w_ih, w_hh, b_ih, b_hh):
    """Host-side shard + transpose + fp8 split. Returns per-core input maps."""
    x = np.asarray(x, np.float32)
    h = np.asarray(h, np.float32)
    c = np.asarray(c, np.float32)
    w_ih = np.asarray(w_ih, np.float32)
    w_hh = np.asarray(w_hh, np.float32)
    bias = np.asarray(b_ih, np.float32) + np.asarray(b_hh, np.float32)

    # Activations: shared by all cores. Planes 0/1 = S/X at scale AQ.
    xS, xX = _quant_pair(x.T, AQ)            # [IN, B]
    hS, hX = _quant_pair(h.T, AQ)            # [H, B]
    xq = _interleave(xS, xX, KX)
    hq = _interleave(hS, hX, KH)
    cTt = np.ascontiguousarray(c.T)          # [H, B]

    # Weights: quantize the full transposed matrices once, slice per core.
    wihR, wihW = _quant_pair(w_ih.T, WQ)     # [IN, 4H] main in wihW
    whhR, whhW = _quant_pair(w_hh.T, WQ)     # [H, 4H]

    # m-tile bias scale: 0.5 for tanh-based sigmoid gates (f,i,o), KAPPA for
    # the sin gate (ic) whose wrap runs on kappa-scaled psum values.
    mscale = np.array([0.5, 0.5, KAPPA, 0.5] * 2, np.float32)

    in_maps = []
    for core in range(NCORES):
        cols = np.concatenate(
            [gate * H + core * SH + half * P + np.arange(P)
             for half in range(2) for gate in range(4)])
        # Full per-core [K=3072, G] main/residual planes, k-tiles x then h.
        Wall = np.concatenate([wihW[:, cols], whhW[:, cols]], axis=0)
        Rall = np.concatenate([wihR[:, cols], whhR[:, cols]], axis=0)
        W4 = Wall.reshape(KT, P, MT, P)              # [kt, p, m, j]
        R4 = Rall.reshape(KT, P, MT, P)
        # SwInterleave stored layout per 256-col block: A/B column-pairs
        # interleaved, columns reversed: st[:,2u]=A[:,127-u], st[:,2u+1]=B.
        wmh = np.empty((KP, P, MT, 2 * P), E4NP)
        wmh[..., 0::2] = W4[0::2][..., ::-1]         # A = W'[2kp]
        wmh[..., 1::2] = W4[1::2][..., ::-1]         # B = W'[2kp+1]
        wch = np.empty((KT, P, MT, 2 * P), E4NP)
        wch[..., 0::2] = W4[..., ::-1]               # A = W'[kt] (pairs S)
        wch[..., 1::2] = R4[..., ::-1]               # B = R[kt]  (pairs X)
        b_c = bias[cols]                             # [G]
        bias_mat = np.ascontiguousarray(
            (b_c.reshape(MT, P) * mscale[:, None]).T)  # [P, MT]
        in_maps.append({
            "xq": xq,
            "hq": hq,
            "cT": np.ascontiguousarray(cTt[core * SH:(core + 1) * SH]),
            # m-major: rows (m, p), window per m contiguous
            "wm": np.ascontiguousarray(
                wmh.transpose(2, 1, 0, 3).reshape(MT * P, KP * 256)),
            "wc": np.ascontiguousarray(
                wch.transpose(2, 1, 0, 3).reshape(MT * P, KT * 256)),
            "biasd": bias_mat,
        })
    return in_maps


def assemble_outputs(results):
    """results: per-core dicts with htT/ctT [SH, B] -> full (ht, ct)."""
    htT = np.concatenate([results[c]["htT"] for c in range(NCORES)], axis=0)
    ctT = np.concatenate([results[c]["ctT"] for c in range(NCORES)], axis=0)
    ht = np.ascontiguousarray(htT.T)
    ct = np.ascontiguousarray(ctT.T)
    return ht, ct


def kernel(x, h, c, w_ih, w_hh, b_ih, b_hh):
    nc = _get_module(repeats=1)
    in_maps = make_in_maps(x, h, c, w_ih, w_hh, b_ih, b_hh)
    res = run_bass_kernel_spmd(nc, in_maps, core_ids=list(range(NCORES)))
    return assemble_outputs(res.results)
